# revision 35
# baseline (speedup 1.0000x reference)
"""Causal self-attention (B=4, L=2048, D=1024, H=16) on 8 Trainium2 NeuronCores.

Sharding: core c handles batch b = c//2 and head-group hg = c%2 (8 of 16 heads).
Each core computes its local QKV projection, causal flash-style attention for
its 8 heads, and a partial output projection against its 512 columns of
proj_w. The host sums the two partial outputs per batch and adds proj_b plus
the v-bias term (proj_w_local.T @ v_bias_local, a constant row — the v bias
contribution commutes through the softmax-normalized PV + projection).

Device layouts (per core):
  xT    [1024, L]   x[b].T              (contraction dim d on partitions)
  w_qk  [1024,1024] qkv_w local q+k rows, transposed; q part pre-scaled by
                    HEAD_DIM**-0.5 (folded into weights+bias)
  w_v   [1024, 512] qkv_w local v rows, transposed
  qkT   [1024, L]   (q;k) features on partitions, tokens on free dim
  vaug  [L, 8*65]   v in token-major layout, one extra ones-column per head
                    (the ones column makes the PV matmul also produce the
                    softmax denominator as psum row 64)
  yT    [512, L]    normalized attention output, features on partitions
  projT [512, 1024] proj_w local columns, transposed

Causal structure is exploited at [128k x 512q] block granularity with
query-width trimming: for a diagonal block ki = 4*qj + j, only query columns
[128*j, 512) are kept (scores matmul, exp, and PV all run on the trimmed
window), and the 0/1 triangle mask multiply is applied with one batched
[128,768] pattern covering both diagonal blocks of a ki-pair.
Softmax skips the max-subtraction (scores are O(+-10) here, far from fp32
overflow) so P = exp(S).

Scheduling notes (these were the big wins over the naive ordering):
- all DMAs are batched (one descriptor-dense dma_start per tensor / per
  l-block output slab) — per-dma_start dispatch on the issuing sequencer
  otherwise serializes the pipeline;
- x and all weights are loaded in the preamble so no in-order DMA-queue
  wait can stall compute;
- emission order is software-pipelined (qkv(lb), proj(lb-1), attn(lb)) so
  QKV(lb+1) psum tiles sit ahead of proj(lb) in the shared psum pool's
  FIFO slot-grant queue and fill PE idle time during attention(lb);
- proj accumulation runs f-descending so its slot-opening matmul waits on
  the last-completed yT tile instead of pinning a psum slot open.
"""

import os
import sys

import numpy as np

for _p in ("/opt/trn_rl_repo", "/root/.axon_site/_ro/trn_rl_repo"):
    if os.path.isdir(_p) and _p not in sys.path:
        sys.path.append(_p)

import ml_dtypes  # noqa: E402
import concourse.bass as bass  # noqa: E402
import concourse.tile as tile  # noqa: E402
from concourse import bacc, mybir  # noqa: E402
from concourse.bass_utils import run_bass_kernel_spmd  # noqa: E402

DIM = 1024
NUM_HEADS = 16
HEAD_DIM = 64
SCALE = HEAD_DIM**-0.5
B = 4
L = 2048
NCORES = 8
HLOC = 8  # heads per core

F32 = mybir.dt.float32
BF16 = mybir.dt.bfloat16

DEFAULT_CFG = ("bf16",) * 5  # kept for test.py compatibility


def schedule_from_mask(am, Lc):
    """Verify the mask is causal-tril and build the trimmed block schedule.

    Returns (sched, patterns): sched[qj] = list of (ki, off) blocks where
    off is the trimmed query-column offset within the 512-wide q-block;
    patterns = [tri] with tri the [128, 128] lower-triangle keep mask laid
    out [k, q] (kept iff q >= k).
    """
    am = np.asarray(am) != 0
    expect = np.tril(np.ones((Lc, Lc), dtype=bool))
    assert am.shape == (Lc, Lc) and bool((am == expect).all()), \
        "kernel specialized for the causal tril mask"
    sched = []
    for qj in range(Lc // 512):
        row = []
        for ki in range(qj * 4 + 4):
            off = max(0, 128 * (ki - 4 * qj))
            row.append((ki, off))
        sched.append(row)
    kk, qq = np.meshgrid(np.arange(128), np.arange(128), indexing="ij")
    tri = (qq >= kk).astype(np.float32)  # [k, q]
    # batched diagonal-pair pattern [128, 768]: a diag pair (j, j+1) has
    # its two triangle regions 640 cols apart in the pt tile; the 512 cols
    # between them multiply by 1 (valid P) or by anything (gap cols that
    # are never read), so one [tri | ones | tri] pattern covers both.
    pat = np.concatenate([tri, np.ones((128, 512), np.float32), tri], 1)
    return sched, [pat]


def build_nc(Lc, sched, n_pat, cfg=DEFAULT_CFG, nrep=1, phases=3):
    """Emit the per-core Bass/Tile program. Same program runs on all cores.

    nrep > 1 wraps the whole body in an on-device For_i loop — used only
    by the dev timing harness to amortize host/dispatch overhead.
    phases: 1 = QKV only, 2 = +attention, 3 = full (dev decomposition).
    """
    NLB = Lc // 512   # l-blocks (also q-blocks)
    NKT = Lc // 128   # k-tiles
    ND = DIM // 128   # contraction tiles for QKV

    nc = bacc.Bacc("TRN2", target_bir_lowering=False, debug=False)

    xT = nc.dram_tensor("xT", [DIM, Lc], BF16, kind="ExternalInput")
    w_qk = nc.dram_tensor("w_qk", [DIM, 1024], BF16, kind="ExternalInput")
    w_v = nc.dram_tensor("w_v", [DIM, 512], BF16, kind="ExternalInput")
    bqk = nc.dram_tensor("bqk", [128, 8], F32, kind="ExternalInput")
    masks = nc.dram_tensor("masks", [1, 128, 768], BF16, kind="ExternalInput")
    projT = nc.dram_tensor("projT", [512, 1024], BF16, kind="ExternalInput")
    y = nc.dram_tensor("y", [Lc, 1024], BF16, kind="ExternalOutput")

    with tile.TileContext(nc) as tc:
        import contextlib
        with contextlib.ExitStack() as ctx:
            sing = ctx.enter_context(tc.tile_pool(name="sing", bufs=1))

            # persistent buffers — q/k/y split per l-block so QKV(lb+1)
            # writes land on different tiles than attention(lb) reads
            # (Tile dep tracking would otherwise serialize the pipeline)
            qkT = [[sing.tile([128, 512], BF16, tag=f"qkT{t}_{b}",
                              name=f"qkT{t}_{b}") for b in range(NLB)]
                   for t in range(8)]
            vaug = [sing.tile([128, HLOC * 65], BF16, tag=f"vaug{t}",
                              name=f"vaug{t}") for t in range(NKT)]
            yT = [[sing.tile([128, 512], BF16, tag=f"yT{t}_{b}",
                             name=f"yT{t}_{b}") for b in range(NLB)]
                  for t in range(4)]
            wqk_all = sing.tile([128, ND * 1024], BF16, tag="wqk",
                                name="wqk")
            projT_all = sing.tile([128, 4 * 1024], BF16, tag="pw", name="pw")
            wv_all = sing.tile([128, ND * 512], BF16, tag="wv", name="wv")
            bqk_sb = sing.tile([128, 8], F32, tag="bqk_sb", name="bqk_sb")
            mask_sb = sing.tile([128, 768], BF16, tag="msk", name="msk")
            wqk_sb = [wqk_all[:, t * 1024:(t + 1) * 1024] for t in range(ND)]
            projT_sb = [projT_all[:, t * 1024:(t + 1) * 1024]
                        for t in range(4)]
            wv_sb = [wv_all[:, t * 512:(t + 1) * 512] for t in range(ND)]

            for t in range(NKT):
                va = vaug[t].rearrange("p (h c) -> p h c", c=65)
                nc.vector.memset(va[:, :, 64:65], 1.0)

            # single batched preamble DMA per input tensor; x is loaded
            # whole up front so no in-order DMA-queue wait can stall the
            # QKV(lb+1) matmuls that overlap attention(lb)
            xsb = [sing.tile([128, ND * 512], BF16, tag=f"x{b}",
                             name=f"x{b}") for b in range(NLB)]
            nc.sync.dma_start(
                wqk_all.rearrange("p (c m) -> p c m", m=1024),
                w_qk.rearrange("(c p) m -> p c m", p=128))
            nc.sync.dma_start(
                wv_all.rearrange("p (c m) -> p c m", m=512),
                w_v.rearrange("(c p) m -> p c m", p=128))
            nc.sync.dma_start(bqk_sb[:, :], bqk[:, :])
            nc.sync.dma_start(mask_sb[:, :], masks[0, :, :])
            for b in range(NLB):
                nc.sync.dma_start(
                    xsb[b].rearrange("p (c j) -> p c j", j=512),
                    xT[:, b * 512:(b + 1) * 512].rearrange(
                        "(c p) j -> p c j", p=128))
            nc.sync.dma_start(
                projT_all.rearrange("p (f e) -> p f e", e=1024),
                projT.rearrange("(f p) e -> p f e", p=128))

            # streaming pools
            ptp = ctx.enter_context(tc.tile_pool(name="ptp", bufs=8))
            osp = ctx.enter_context(tc.tile_pool(name="osp", bufs=5))
            rsp = ctx.enter_context(tc.tile_pool(name="rsp", bufs=6))
            repp = ctx.enter_context(tc.tile_pool(name="repp", bufs=5))
            outp = ctx.enter_context(tc.tile_pool(name="outp", bufs=3))
            # separate psum pools so QKV/proj matmuls (1-bank tiles) never
            # wait on score-slot frees: 2x1 (qkv/proj) + 2x2 (scores)
            # + 2x1 (pov) = 8 banks.
            pmm = ctx.enter_context(tc.tile_pool(name="pmm", bufs=2,
                                                 space="PSUM"))
            psc = ctx.enter_context(tc.tile_pool(name="psc", bufs=2,
                                                 space="PSUM"))
            pov = ctx.enter_context(tc.tile_pool(name="pov", bufs=2,
                                                 space="PSUM"))

            rep_ctx = (tc.For_i(0, nrep, 1) if nrep > 1
                       else contextlib.nullcontext())
            with rep_ctx:
                def emit_qkv(lb):
                    xt = [xsb[lb][:, d * 512:(d + 1) * 512]
                          for d in range(ND)]
                    for e in range(8):
                        ps = pmm.tile([128, 512], F32, tag="ps",
                                      name=f"q{lb}_{e}")
                        for d in range(ND):
                            nc.tensor.matmul(
                                ps[:, :],
                                lhsT=wqk_sb[d][:, e * 128:(e + 1) * 128],
                                rhs=xt[d][:, :],
                                start=(d == 0), stop=(d == ND - 1))
                        nc.vector.tensor_scalar_add(
                            out=qkT[e][lb][:, :], in0=ps[:, :],
                            scalar1=bqk_sb[:, e:e + 1])
                    for ls in range(4):
                        lt = lb * 4 + ls
                        ps = pmm.tile([128, 512], F32, tag="ps", name=f"v{lt}")
                        for d in range(ND):
                            nc.tensor.matmul(
                                ps[:, :],
                                lhsT=xt[d][:, ls * 128:(ls + 1) * 128],
                                rhs=wv_sb[d][:, :],
                                start=(d == 0), stop=(d == ND - 1))
                        dst = vaug[lt].rearrange(
                            "p (h c) -> p h c", c=65)[:, :, 0:64]
                        src = ps.rearrange("p (h c) -> p h c", c=64)
                        nc.vector.tensor_copy(dst, src)

                # ---- causal attention for q-block qj ----------------------
                # heads run in (even, odd) pairs: even heads sit at SBUF
                # partitions 0-63, odd at 64-127, so their K=64 score
                # matmuls land in different PE row groups and run
                # concurrently (issue order alternates heads to overlap).
                # Two ki-tiles share one 2-bank psum so exp batches up to
                # [128, 1024].
                def emit_attn(qj):
                    lb = qj
                    blocks = sched[qj]
                    pairs = [blocks[i:i + 2] for i in range(0, len(blocks), 2)]
                    for hp in range(HLOC // 2):
                        t = hp
                        h0, h1 = 2 * hp, 2 * hp + 1
                        po = {}
                        for h in (h0, h1):
                            po[h] = pov.tile([65, 512], F32, tag="po",
                                             name=f"o{qj}_{h}")
                        first = True
                        for pi, pair in enumerate(pairs):
                            ps, pt = {}, {}
                            for h in (h0, h1):
                                ps[h] = psc.tile([128, 1024], F32, tag="ps",
                                                 name=f"s{qj}_{h}_{pi}")
                            # scores: interleave heads so row-group-disjoint
                            # matmuls sit adjacent in PE issue order
                            for j, (ki, off) in enumerate(pair):
                                for h in (h0, h1):
                                    base = (h % 2) * 64
                                    nc.tensor.matmul(
                                        ps[h][:, j * 512 + off:(j + 1) * 512],
                                        lhsT=qkT[4 + t][ki // 4][
                                            base:base + 64,
                                            (ki % 4) * 128:(ki % 4 + 1) * 128],
                                        rhs=qkT[t][lb][base:base + 64,
                                                       off:512],
                                        start=True, stop=True)
                            # single exp per (head, pair) spanning the trim
                            # gap (garbage cols are cheaper than a second
                            # ACT dispatch and are never read downstream);
                            # diagonal pairs get one batched [128,768]
                            # triangle-mask multiply covering both blocks
                            o0 = pair[0][1]
                            diag = pair[0][0] >= 4 * qj
                            for h in (h0, h1):
                                pt[h] = ptp.tile([128, 1024], BF16, tag="pt",
                                                 name=f"p{qj}_{h}_{pi}")
                                nc.scalar.activation(
                                    out=pt[h][:, o0:1024],
                                    in_=ps[h][:, o0:1024],
                                    func=mybir.ActivationFunctionType.Exp)
                                if diag:
                                    nc.vector.tensor_mul(
                                        pt[h][:, o0:o0 + 768],
                                        pt[h][:, o0:o0 + 768],
                                        mask_sb[:, :])
                            for j, (ki, off) in enumerate(pair):
                                for h in (h0, h1):
                                    nc.tensor.matmul(
                                        po[h][:, off:512],
                                        lhsT=vaug[ki][:, h * 65:(h + 1) * 65],
                                        rhs=pt[h][:, j * 512 + off:
                                                  (j + 1) * 512],
                                        start=first,
                                        stop=(pi == len(pairs) - 1
                                              and j == len(pair) - 1))
                                first = False
                        for h in (h0, h1):
                            base = (h % 2) * 64
                            # single 65-row copy so the po psum slot frees
                            # after one op; recip then reads the SBUF copy
                            osb = osp.tile([65, 512], F32, tag="osb",
                                           name=f"ob{qj}_{h}")
                            nc.vector.tensor_copy(osb[:, :], po[h][0:65, :])
                            rsb = rsp.tile([1, 512], F32, tag="rsb",
                                           name=f"r{qj}_{h}")
                            nc.vector.reciprocal(rsb[:, :], osb[64:65, :])
                            rep = repp.tile([64, 512], F32, tag="rep",
                                            name=f"rp{qj}_{h}")
                            nc.gpsimd.partition_broadcast(
                                rep[:, :], rsb[:, :], channels=64)
                            nc.vector.tensor_mul(
                                yT[t][lb][base:base + 64, :],
                                osb[0:64, :], rep[:, :])

                # ---- output projection for one q-block --------------------
                # partial sums stage into one [128, 4096] SBUF tile so a
                # single DMA writes the whole 512-token output slab
                def emit_proj(qj):
                    lb = qj
                    l0 = qj * 512
                    yob = outp.tile([128, 4 * 1024], BF16, tag="ob",
                                    name=f"yo{qj}")
                    for ls in range(4):
                        for e2 in range(2):
                            ps = pmm.tile([128, 512], F32, tag="ps",
                                          name=f"pj{qj}_{e2}_{ls}")
                            # f descending: the slot-opening matmul waits on
                            # the LAST-completed yT tile, so the psum slot is
                            # held briefly instead of pinning the pool (which
                            # would block QKV(lb+1) from overlapping)
                            for i, f in enumerate((3, 2, 1, 0)):
                                nc.tensor.matmul(
                                    ps[:, :],
                                    lhsT=yT[f][lb][:, ls * 128:
                                                   (ls + 1) * 128],
                                    rhs=projT_sb[f][:, e2 * 512:
                                                    (e2 + 1) * 512],
                                    start=(i == 0), stop=(i == 3))
                            nc.vector.tensor_copy(
                                yob[:, ls * 1024 + e2 * 512:
                                    ls * 1024 + e2 * 512 + 512], ps[:, :])
                    nc.sync.dma_start(
                        y[l0:l0 + 512, :].rearrange("(ls p) e -> p ls e",
                                                    p=128),
                        yob.rearrange("p (ls e) -> p ls e", e=1024))

                # software-pipelined emission: proj(lb-1) is emitted AFTER
                # qkv(lb) so qkv(lb+1)'s psum tiles are ahead of proj(lb)'s
                # in the shared pool's slot-grant queue — this is what lets
                # the next block's QKV matmuls fill PE idle time during
                # attention(lb) instead of queuing behind proj
                for lb in range(NLB):
                    emit_qkv(lb)
                    if phases >= 3 and lb >= 1:
                        emit_proj(lb - 1)
                    if phases >= 2:
                        emit_attn(lb)
                if phases >= 3:
                    emit_proj(NLB - 1)
    return nc


def make_core_inputs(x, attn_mask, qkv_w, qkv_b, proj_w, patterns,
                     cfg=DEFAULT_CFG, Lc=L):
    """Host-side shard prep: per-core input dicts for cores 0..7."""
    bf = ml_dtypes.bfloat16
    mask_arr = np.zeros((1, 128, 768), np.float32)
    mask_arr[0] = patterns[0]
    mask_arr = mask_arr.astype(bf)

    in_maps = []
    shared = {}
    for c in range(NCORES):
        b, hg = c // 2, c % 2
        if b not in shared:
            shared[b] = np.ascontiguousarray(
                np.asarray(x[b], np.float32).T).astype(bf)
        key = ("w", hg)
        if key not in shared:
            rq = qkv_w[hg * 512:hg * 512 + 512, :] * SCALE
            rk = qkv_w[1024 + hg * 512:1024 + hg * 512 + 512, :]
            rv = qkv_w[2048 + hg * 512:2048 + hg * 512 + 512, :]
            w_qk_h = np.ascontiguousarray(
                np.concatenate([rq, rk], 0).T).astype(bf)
            w_v_h = np.ascontiguousarray(rv.T).astype(bf)
            bq = qkv_b[hg * 512:hg * 512 + 512] * SCALE
            bk = qkv_b[1024 + hg * 512:1024 + hg * 512 + 512]
            bqk_h = np.ascontiguousarray(
                np.concatenate([bq, bk]).reshape(8, 128).T).astype(np.float32)
            projT_h = np.ascontiguousarray(
                proj_w[:, hg * 512:hg * 512 + 512].T).astype(bf)
            shared[key] = (w_qk_h, w_v_h, bqk_h, projT_h)
        w_qk_h, w_v_h, bqk_h, projT_h = shared[("w", hg)]
        in_maps.append({
            "xT": shared[b],
            "w_qk": w_qk_h,
            "w_v": w_v_h,
            "bqk": bqk_h,
            "masks": mask_arr,
            "projT": projT_h,
        })
    return in_maps


_NC_CACHE = {}
LAST_RESULTS = None


def kernel(**inputs):
    x = np.asarray(inputs["x"], np.float32)
    attn_mask = np.asarray(inputs["attn_mask"])
    qkv_w = np.asarray(inputs["qkv_w"], np.float32)
    qkv_b = np.asarray(inputs["qkv_b"], np.float32)
    proj_w = np.asarray(inputs["proj_w"], np.float32)
    proj_b = np.asarray(inputs["proj_b"], np.float32)

    sched, patterns = schedule_from_mask(attn_mask, L)

    key = (L, tuple(tuple(r) for r in sched))
    if key not in _NC_CACHE:
        nc = build_nc(L, sched, len(patterns))
        if not nc.is_finalized():
            nc.finalize()  # bacc regalloc etc.; bass2jax serializes as-is
        _NC_CACHE[key] = nc
    nc = _NC_CACHE[key]

    in_maps = make_core_inputs(x, attn_mask, qkv_w, qkv_b, proj_w, patterns)
    res = run_bass_kernel_spmd(nc, in_maps, list(range(NCORES)))
    global LAST_RESULTS
    LAST_RESULTS = res

    # host epilogue: sum per-batch partials, add proj_b and the v-bias fold
    # (y_partial excludes v bias; its contribution is proj_local.T @ bv_local,
    # constant across tokens because softmax rows sum to 1)
    bv = qkv_b[2048:3072]
    const = proj_w @ bv  # [1024] == sum_hg proj_w[:, hg slice] @ bv[hg slice]
    out = np.empty((B, L, DIM), np.float32)
    for b in range(B):
        out[b] = (res.results[2 * b]["y"].astype(np.float32)
                  + res.results[2 * b + 1]["y"].astype(np.float32)
                  + proj_b + const)
    return out


# revision 36
# speedup vs baseline: 1.1727x; 1.1727x over previous
"""Causal self-attention (B=4, L=2048, D=1024, H=16) on 8 Trainium2 NeuronCores.

Sharding: core c handles batch b = c//2 and head-group hg = c%2 (8 of 16 heads).
Each core computes its local QKV projection, causal flash-style attention for
its 8 heads, and a partial output projection against its 512 columns of
proj_w. The host sums the two partial outputs per batch and adds proj_b plus
the v-bias term (proj_w_local.T @ v_bias_local, a constant row — the v bias
contribution commutes through the softmax-normalized PV + projection).

Device layouts (per core):
  xT    [1024, L]   x[b].T              (contraction dim d on partitions)
  w_qk  [1024,1024] qkv_w local q+k rows, transposed; q part pre-scaled by
                    HEAD_DIM**-0.5 (folded into weights+bias)
  w_v   [1024, 512] qkv_w local v rows, transposed
  qkT   [1024, L]   (q;k) features on partitions, tokens on free dim
  vaug  [L, 8*65]   v in token-major layout, one extra ones-column per head
                    (the ones column makes the PV matmul also produce the
                    softmax denominator as psum row 64)
  yT    [512, L]    normalized attention output, features on partitions
  projT [512, 1024] proj_w local columns, transposed

Causal structure is exploited at [128k x 512q] block granularity with
query-width trimming: for a diagonal block ki = 4*qj + j, only query columns
[128*j, 512) are kept (scores matmul, exp, and PV all run on the trimmed
window), and the 0/1 triangle mask multiply is applied with one batched
[128,768] pattern covering both diagonal blocks of a ki-pair.
Softmax skips the max-subtraction (scores are O(+-10) here, far from fp32
overflow) so P = exp(S).

Scheduling notes (these were the big wins over the naive ordering):
- all DMAs are batched (one descriptor-dense dma_start per tensor / per
  l-block output slab) — per-dma_start dispatch on the issuing sequencer
  otherwise serializes the pipeline;
- x and all weights are loaded in the preamble so no in-order DMA-queue
  wait can stall compute;
- emission order is software-pipelined (qkv(lb), proj(lb-1), attn(lb)) so
  QKV(lb+1) psum tiles sit ahead of proj(lb) in the shared psum pool's
  FIFO slot-grant queue and fill PE idle time during attention(lb);
- proj accumulation runs f-descending so its slot-opening matmul waits on
  the last-completed yT tile instead of pinning a psum slot open.
"""

import os
import sys

import numpy as np

for _p in ("/opt/trn_rl_repo", "/root/.axon_site/_ro/trn_rl_repo"):
    if os.path.isdir(_p) and _p not in sys.path:
        sys.path.append(_p)

import ml_dtypes  # noqa: E402
import concourse.bass as bass  # noqa: E402
import concourse.tile as tile  # noqa: E402
from concourse import bacc, mybir  # noqa: E402
from concourse.bass_utils import run_bass_kernel_spmd  # noqa: E402

DIM = 1024
NUM_HEADS = 16
HEAD_DIM = 64
SCALE = HEAD_DIM**-0.5
B = 4
L = 2048
NCORES = 8
HLOC = 8  # heads per core

F32 = mybir.dt.float32
BF16 = mybir.dt.bfloat16

DEFAULT_CFG = ("bf16",) * 5  # kept for test.py compatibility


def schedule_from_mask(am, Lc):
    """Verify the mask is causal-tril and build the trimmed block schedule.

    Returns (sched, patterns): sched[qj] = list of (ki, off) blocks where
    off is the trimmed query-column offset within the 512-wide q-block;
    patterns = [tri] with tri the [128, 128] lower-triangle keep mask laid
    out [k, q] (kept iff q >= k).
    """
    am = np.asarray(am) != 0
    expect = np.tril(np.ones((Lc, Lc), dtype=bool))
    assert am.shape == (Lc, Lc) and bool((am == expect).all()), \
        "kernel specialized for the causal tril mask"
    sched = []
    for qj in range(Lc // 512):
        row = []
        for ki in range(qj * 4 + 4):
            off = max(0, 128 * (ki - 4 * qj))
            row.append((ki, off))
        sched.append(row)
    kk, qq = np.meshgrid(np.arange(128), np.arange(128), indexing="ij")
    tri = (qq >= kk).astype(np.float32)  # [k, q]
    # batched diagonal-pair pattern [128, 768]: a diag pair (j, j+1) has
    # its two triangle regions 640 cols apart in the pt tile; the 512 cols
    # between them multiply by 1 (valid P) or by anything (gap cols that
    # are never read), so one [tri | ones | tri] pattern covers both.
    pat = np.concatenate([tri, np.ones((128, 512), np.float32), tri], 1)
    return sched, [pat]


def build_nc(Lc, sched, n_pat, cfg=DEFAULT_CFG, nrep=1, phases=3):
    """Emit the per-core Bass/Tile program. Same program runs on all cores.

    nrep > 1 wraps the whole body in an on-device For_i loop — used only
    by the dev timing harness to amortize host/dispatch overhead.
    phases: 1 = QKV only, 2 = +attention, 3 = full (dev decomposition).
    """
    NLB = Lc // 512   # l-blocks (also q-blocks)
    NKT = Lc // 128   # k-tiles
    ND = DIM // 128   # contraction tiles for QKV

    nc = bacc.Bacc("TRN2", target_bir_lowering=False, debug=False)

    xT = nc.dram_tensor("xT", [DIM, Lc], BF16, kind="ExternalInput")
    w_qk = nc.dram_tensor("w_qk", [DIM, 1024], BF16, kind="ExternalInput")
    w_v = nc.dram_tensor("w_v", [DIM, 512], BF16, kind="ExternalInput")
    bqk = nc.dram_tensor("bqk", [128, 8], F32, kind="ExternalInput")
    masks = nc.dram_tensor("masks", [1, 128, 768], BF16, kind="ExternalInput")
    projT = nc.dram_tensor("projT", [512, 1024], BF16, kind="ExternalInput")
    y = nc.dram_tensor("y", [Lc, 1024], BF16, kind="ExternalOutput")

    with tile.TileContext(nc) as tc:
        import contextlib
        with contextlib.ExitStack() as ctx:
            sing = ctx.enter_context(tc.tile_pool(name="sing", bufs=1))

            # persistent buffers — q/k/y split per l-block so QKV(lb+1)
            # writes land on different tiles than attention(lb) reads
            # (Tile dep tracking would otherwise serialize the pipeline)
            qkT = [[sing.tile([128, 512], BF16, tag=f"qkT{t}_{b}",
                              name=f"qkT{t}_{b}") for b in range(NLB)]
                   for t in range(8)]
            vaug = [sing.tile([128, HLOC * 65], BF16, tag=f"vaug{t}",
                              name=f"vaug{t}") for t in range(NKT)]
            yT = [[sing.tile([128, 512], BF16, tag=f"yT{t}_{b}",
                             name=f"yT{t}_{b}") for b in range(NLB)]
                  for t in range(4)]
            wqk_all = sing.tile([128, ND * 1024], BF16, tag="wqk",
                                name="wqk")
            projT_all = sing.tile([128, 4 * 1024], BF16, tag="pw", name="pw")
            wv_all = sing.tile([128, ND * 512], BF16, tag="wv", name="wv")
            bqk_sb = sing.tile([128, 8], F32, tag="bqk_sb", name="bqk_sb")
            mask_sb = sing.tile([128, 768], BF16, tag="msk", name="msk")
            wqk_sb = [wqk_all[:, t * 1024:(t + 1) * 1024] for t in range(ND)]
            projT_sb = [projT_all[:, t * 1024:(t + 1) * 1024]
                        for t in range(4)]
            wv_sb = [wv_all[:, t * 512:(t + 1) * 512] for t in range(ND)]

            for t in range(NKT):
                va = vaug[t].rearrange("p (h c) -> p h c", c=65)
                nc.vector.memset(va[:, :, 64:65], 1.0)

            # single batched preamble DMA per input tensor; x is loaded
            # whole up front so no in-order DMA-queue wait can stall the
            # QKV(lb+1) matmuls that overlap attention(lb)
            xsb = [sing.tile([128, ND * 512], BF16, tag=f"x{b}",
                             name=f"x{b}") for b in range(NLB)]
            nc.sync.dma_start(
                wqk_all.rearrange("p (c m) -> p c m", m=1024),
                w_qk.rearrange("(c p) m -> p c m", p=128))
            nc.sync.dma_start(
                wv_all.rearrange("p (c m) -> p c m", m=512),
                w_v.rearrange("(c p) m -> p c m", p=128))
            nc.sync.dma_start(bqk_sb[:, :], bqk[:, :])
            nc.sync.dma_start(mask_sb[:, :], masks[0, :, :])
            for b in range(NLB):
                nc.sync.dma_start(
                    xsb[b].rearrange("p (c j) -> p c j", j=512),
                    xT[:, b * 512:(b + 1) * 512].rearrange(
                        "(c p) j -> p c j", p=128))
            nc.sync.dma_start(
                projT_all.rearrange("p (f e) -> p f e", e=1024),
                projT.rearrange("(f p) e -> p f e", p=128))

            # streaming pools
            ptp = ctx.enter_context(tc.tile_pool(name="ptp", bufs=8))
            osp = ctx.enter_context(tc.tile_pool(name="osp", bufs=5))
            rsp = ctx.enter_context(tc.tile_pool(name="rsp", bufs=6))
            repp = ctx.enter_context(tc.tile_pool(name="repp", bufs=5))
            outp = ctx.enter_context(tc.tile_pool(name="outp", bufs=3))
            # separate psum pools so QKV/proj matmuls (1-bank tiles) never
            # wait on score-slot frees: 2x1 (qkv/proj) + 2x2 (scores)
            # + 2x1 (pov) = 8 banks.
            pmm = ctx.enter_context(tc.tile_pool(name="pmm", bufs=2,
                                                 space="PSUM"))
            psc = ctx.enter_context(tc.tile_pool(name="psc", bufs=2,
                                                 space="PSUM"))
            pov = ctx.enter_context(tc.tile_pool(name="pov", bufs=2,
                                                 space="PSUM"))

            rep_ctx = (tc.For_i(0, nrep, 1) if nrep > 1
                       else contextlib.nullcontext())
            with rep_ctx:
                def emit_qkv(lb):
                    xt = [xsb[lb][:, d * 512:(d + 1) * 512]
                          for d in range(ND)]
                    for e in range(8):
                        ps = pmm.tile([128, 512], F32, tag="ps",
                                      name=f"q{lb}_{e}")
                        for d in range(ND):
                            nc.tensor.matmul(
                                ps[:, :],
                                lhsT=wqk_sb[d][:, e * 128:(e + 1) * 128],
                                rhs=xt[d][:, :],
                                start=(d == 0), stop=(d == ND - 1))
                        nc.vector.tensor_scalar_add(
                            out=qkT[e][lb][:, :], in0=ps[:, :],
                            scalar1=bqk_sb[:, e:e + 1])
                    for ls in range(4):
                        lt = lb * 4 + ls
                        ps = pmm.tile([128, 512], F32, tag="ps", name=f"v{lt}")
                        for d in range(ND):
                            nc.tensor.matmul(
                                ps[:, :],
                                lhsT=xt[d][:, ls * 128:(ls + 1) * 128],
                                rhs=wv_sb[d][:, :],
                                start=(d == 0), stop=(d == ND - 1))
                        dst = vaug[lt].rearrange(
                            "p (h c) -> p h c", c=65)[:, :, 0:64]
                        src = ps.rearrange("p (h c) -> p h c", c=64)
                        nc.vector.tensor_copy(dst, src)

                # ---- causal attention for q-block qj ----------------------
                # heads run in (even, odd) pairs: even heads sit at SBUF
                # partitions 0-63, odd at 64-127, so their K=64 score
                # matmuls land in different PE row groups and run
                # concurrently (issue order alternates heads to overlap).
                # Two ki-tiles share one 2-bank psum so exp batches up to
                # [128, 1024].
                def emit_attn(qj):
                    lb = qj
                    blocks = sched[qj]
                    pairs = [blocks[i:i + 2] for i in range(0, len(blocks), 2)]
                    for hp in range(HLOC // 2):
                        t = hp
                        h0, h1 = 2 * hp, 2 * hp + 1
                        po = {}
                        for h in (h0, h1):
                            po[h] = pov.tile([65, 512], F32, tag="po",
                                             name=f"o{qj}_{h}")
                        first = True
                        for pi, pair in enumerate(pairs):
                            ps, pt = {}, {}
                            for h in (h0, h1):
                                ps[h] = psc.tile([128, 1024], F32, tag="ps",
                                                 name=f"s{qj}_{h}_{pi}")
                            # scores: interleave heads so row-group-disjoint
                            # matmuls sit adjacent in PE issue order
                            for j, (ki, off) in enumerate(pair):
                                for h in (h0, h1):
                                    base = (h % 2) * 64
                                    nc.tensor.matmul(
                                        ps[h][:, j * 512 + off:(j + 1) * 512],
                                        lhsT=qkT[4 + t][ki // 4][
                                            base:base + 64,
                                            (ki % 4) * 128:(ki % 4 + 1) * 128],
                                        rhs=qkT[t][lb][base:base + 64,
                                                       off:512],
                                        start=True, stop=True)
                            # single exp per (head, pair) spanning the trim
                            # gap (garbage cols are cheaper than a second
                            # ACT dispatch and are never read downstream);
                            # diagonal pairs get one batched [128,768]
                            # triangle-mask multiply covering both blocks
                            o0 = pair[0][1]
                            diag = pair[0][0] >= 4 * qj
                            for h in (h0, h1):
                                pt[h] = ptp.tile([128, 1024], BF16, tag="pt",
                                                 name=f"p{qj}_{h}_{pi}")
                                nc.scalar.activation(
                                    out=pt[h][:, o0:1024],
                                    in_=ps[h][:, o0:1024],
                                    func=mybir.ActivationFunctionType.Exp)
                                if diag:
                                    nc.vector.tensor_mul(
                                        pt[h][:, o0:o0 + 768],
                                        pt[h][:, o0:o0 + 768],
                                        mask_sb[:, :])
                            for j, (ki, off) in enumerate(pair):
                                for h in (h0, h1):
                                    nc.tensor.matmul(
                                        po[h][:, off:512],
                                        lhsT=vaug[ki][:, h * 65:(h + 1) * 65],
                                        rhs=pt[h][:, j * 512 + off:
                                                  (j + 1) * 512],
                                        start=first,
                                        stop=(pi == len(pairs) - 1
                                              and j == len(pair) - 1))
                                first = False
                        for h in (h0, h1):
                            base = (h % 2) * 64
                            # single 65-row copy so the po psum slot frees
                            # after one op; recip then reads the SBUF copy
                            osb = osp.tile([65, 512], F32, tag="osb",
                                           name=f"ob{qj}_{h}")
                            nc.vector.tensor_copy(osb[:, :], po[h][0:65, :])
                            rsb = rsp.tile([1, 512], F32, tag="rsb",
                                           name=f"r{qj}_{h}")
                            nc.vector.reciprocal(rsb[:, :], osb[64:65, :])
                            rep = repp.tile([64, 512], F32, tag="rep",
                                            name=f"rp{qj}_{h}")
                            nc.gpsimd.partition_broadcast(
                                rep[:, :], rsb[:, :], channels=64)
                            nc.vector.tensor_mul(
                                yT[t][lb][base:base + 64, :],
                                osb[0:64, :], rep[:, :])

                # ---- output projection for one q-block --------------------
                # partial sums stage into one [128, 4096] SBUF tile so a
                # single DMA writes the whole 512-token output slab
                def emit_proj(qj):
                    lb = qj
                    l0 = qj * 512
                    yob = outp.tile([128, 4 * 1024], BF16, tag="ob",
                                    name=f"yo{qj}")
                    for ls in range(4):
                        for e2 in range(2):
                            ps = pmm.tile([128, 512], F32, tag="ps",
                                          name=f"pj{qj}_{e2}_{ls}")
                            # f descending: the slot-opening matmul waits on
                            # the LAST-completed yT tile, so the psum slot is
                            # held briefly instead of pinning the pool (which
                            # would block QKV(lb+1) from overlapping)
                            for i, f in enumerate((3, 2, 1, 0)):
                                nc.tensor.matmul(
                                    ps[:, :],
                                    lhsT=yT[f][lb][:, ls * 128:
                                                   (ls + 1) * 128],
                                    rhs=projT_sb[f][:, e2 * 512:
                                                    (e2 + 1) * 512],
                                    start=(i == 0), stop=(i == 3))
                            nc.vector.tensor_copy(
                                yob[:, ls * 1024 + e2 * 512:
                                    ls * 1024 + e2 * 512 + 512], ps[:, :])
                    nc.sync.dma_start(
                        y[l0:l0 + 512, :].rearrange("(ls p) e -> p ls e",
                                                    p=128),
                        yob.rearrange("p (ls e) -> p ls e", e=1024))

                # software-pipelined emission: proj(lb-1) is emitted AFTER
                # qkv(lb) so qkv(lb+1)'s psum tiles are ahead of proj(lb)'s
                # in the shared pool's slot-grant queue — this is what lets
                # the next block's QKV matmuls fill PE idle time during
                # attention(lb) instead of queuing behind proj
                for lb in range(NLB):
                    emit_qkv(lb)
                    if phases >= 2:
                        emit_attn(lb)
                    if phases >= 3 and lb >= 1:
                        emit_proj(lb - 1)
                if phases >= 3:
                    emit_proj(NLB - 1)
    return nc


def make_core_inputs(x, attn_mask, qkv_w, qkv_b, proj_w, patterns,
                     cfg=DEFAULT_CFG, Lc=L):
    """Host-side shard prep: per-core input dicts for cores 0..7."""
    bf = ml_dtypes.bfloat16
    mask_arr = np.zeros((1, 128, 768), np.float32)
    mask_arr[0] = patterns[0]
    mask_arr = mask_arr.astype(bf)

    in_maps = []
    shared = {}
    for c in range(NCORES):
        b, hg = c // 2, c % 2
        if b not in shared:
            shared[b] = np.ascontiguousarray(
                np.asarray(x[b], np.float32).T).astype(bf)
        key = ("w", hg)
        if key not in shared:
            rq = qkv_w[hg * 512:hg * 512 + 512, :] * SCALE
            rk = qkv_w[1024 + hg * 512:1024 + hg * 512 + 512, :]
            rv = qkv_w[2048 + hg * 512:2048 + hg * 512 + 512, :]
            w_qk_h = np.ascontiguousarray(
                np.concatenate([rq, rk], 0).T).astype(bf)
            w_v_h = np.ascontiguousarray(rv.T).astype(bf)
            bq = qkv_b[hg * 512:hg * 512 + 512] * SCALE
            bk = qkv_b[1024 + hg * 512:1024 + hg * 512 + 512]
            bqk_h = np.ascontiguousarray(
                np.concatenate([bq, bk]).reshape(8, 128).T).astype(np.float32)
            projT_h = np.ascontiguousarray(
                proj_w[:, hg * 512:hg * 512 + 512].T).astype(bf)
            shared[key] = (w_qk_h, w_v_h, bqk_h, projT_h)
        w_qk_h, w_v_h, bqk_h, projT_h = shared[("w", hg)]
        in_maps.append({
            "xT": shared[b],
            "w_qk": w_qk_h,
            "w_v": w_v_h,
            "bqk": bqk_h,
            "masks": mask_arr,
            "projT": projT_h,
        })
    return in_maps


_NC_CACHE = {}
LAST_RESULTS = None


def kernel(**inputs):
    x = np.asarray(inputs["x"], np.float32)
    attn_mask = np.asarray(inputs["attn_mask"])
    qkv_w = np.asarray(inputs["qkv_w"], np.float32)
    qkv_b = np.asarray(inputs["qkv_b"], np.float32)
    proj_w = np.asarray(inputs["proj_w"], np.float32)
    proj_b = np.asarray(inputs["proj_b"], np.float32)

    sched, patterns = schedule_from_mask(attn_mask, L)

    key = (L, tuple(tuple(r) for r in sched))
    if key not in _NC_CACHE:
        nc = build_nc(L, sched, len(patterns))
        if not nc.is_finalized():
            nc.finalize()  # bacc regalloc etc.; bass2jax serializes as-is
        _NC_CACHE[key] = nc
    nc = _NC_CACHE[key]

    in_maps = make_core_inputs(x, attn_mask, qkv_w, qkv_b, proj_w, patterns)
    res = run_bass_kernel_spmd(nc, in_maps, list(range(NCORES)))
    global LAST_RESULTS
    LAST_RESULTS = res

    # host epilogue: sum per-batch partials, add proj_b and the v-bias fold
    # (y_partial excludes v bias; its contribution is proj_local.T @ bv_local,
    # constant across tokens because softmax rows sum to 1)
    bv = qkv_b[2048:3072]
    const = proj_w @ bv  # [1024] == sum_hg proj_w[:, hg slice] @ bv[hg slice]
    out = np.empty((B, L, DIM), np.float32)
    for b in range(B):
        out[b] = (res.results[2 * b]["y"].astype(np.float32)
                  + res.results[2 * b + 1]["y"].astype(np.float32)
                  + proj_b + const)
    return out
